# revision 10
# baseline (speedup 1.0000x reference)
"""Trainium2 Bass kernel for nn_AmidePredictor (3x GAT towers + MLP head).

Strategy (8 NeuronCores, SPMD):
  - Each core owns NB=512 query nodes of every tower (row-sharded N x N attention).
  - Per layer: each core computes its h-block + score columns, AllGathers
    h(+s2) across the 8 cores, then does its 512-row slice of the
    exp(leaky(s1_i+s2_j)) softmax via DVE/ACT and accumulates
    numerator (h-weighted) + denominator on the TensorEngine in bf16.
  - The three towers are interleaved so each tower's AllGather overlaps the
    other towers' compute.
  - Node-sums (free-axis reduce in the transposed layout), one tiny final
    AllGather, then the replicated 385-wide MLP head on every core.

kernel(**inputs) takes the FULL inputs from setup_inputs() and returns the
FULL [1] output. Self-contained: only needs the concourse/bass runtime
(/opt/trn_rl_repo) and the 8 axon NeuronCores.
"""
import os
import sys

for _p in ("/opt/trn_rl_repo",):
    if _p not in sys.path and os.path.isdir(_p):
        sys.path.insert(0, _p)

import numpy as np
import ml_dtypes

N = 4096
D = 128
L = 3
NCORES = 8
NB = N // NCORES          # 512 own query nodes per core
KC = N // 128             # 32 key chunks of 128
MC = NB // 128            # 4 own-node chunks of 128
TOWERS = ("acid", "amine", "int")
HP = 512                  # padded head width (385 -> 512)

EGRP = 8                    # k-chunks per exp/wbig group tile
NPRELU = 2                  # chunks per group handled by ACT Prelu (rest: DVE)
NGRP = KC // EGRP           # groups per layer-tower

LAST_RESULT = None
_CACHED = None


def _build():
    import concourse.bass as bass
    import concourse.bacc as bacc
    import concourse.tile as tile
    import concourse.mybir as mybir

    F32 = mybir.dt.float32
    BF16 = mybir.dt.bfloat16
    AF = mybir.ActivationFunctionType
    ALU = mybir.AluOpType
    AX = mybir.AxisListType
    RG = [list(range(NCORES))]

    nc = bacc.Bacc("TRN2", target_bir_lowering=False, debug=False,
                   num_devices=NCORES)

    ins = {}
    for g in TOWERS:
        ins[f"x0T_{g}"] = nc.dram_tensor(f"x0T_{g}", [D, NB], BF16, kind="ExternalInput")
        ins[f"W_{g}"] = nc.dram_tensor(f"W_{g}", [L, D, D], BF16, kind="ExternalInput")
        ins[f"wf_{g}"] = nc.dram_tensor(f"wf_{g}", [L, D, 2], BF16, kind="ExternalInput")
    ins["ctrl8"] = nc.dram_tensor("ctrl8", [1], F32, kind="ExternalInput")
    ins["Wh0"] = nc.dram_tensor("Wh0", [HP, HP], F32, kind="ExternalInput")
    ins["bh0"] = nc.dram_tensor("bh0", [HP], F32, kind="ExternalInput")
    ins["Wh1"] = nc.dram_tensor("Wh1", [HP, HP], F32, kind="ExternalInput")
    ins["bh1"] = nc.dram_tensor("bh1", [HP], F32, kind="ExternalInput")
    ins["Wpj"] = nc.dram_tensor("Wpj", [HP, D], F32, kind="ExternalInput")
    ins["bpj"] = nc.dram_tensor("bpj", [D], F32, kind="ExternalInput")
    ins["Wo"] = nc.dram_tensor("Wo", [D, 1], F32, kind="ExternalInput")
    ins["bo"] = nc.dram_tensor("bo", [1], F32, kind="ExternalInput")
    out_d = nc.dram_tensor("out", [1], F32, kind="ExternalOutput")

    with tile.TileContext(nc) as tc:
        with tc.tile_pool(name="cpool", bufs=1) as cpool, \
             tc.tile_pool(name="work", bufs=1) as work, \
             tc.tile_pool(name="hpool", bufs=1) as hpool, \
             tc.tile_pool(name="wpool", bufs=1) as wpool, \
             tc.tile_pool(name="psum", bufs=1, space=bass.MemorySpace.PSUM) as psum, \
             tc.tile_pool(name="dpool", bufs=1, space="DRAM") as dpool:

            ones_col = cpool.tile([128, 1], BF16, name="ones_col")
            nc.vector.memset(ones_col[:], 1.0)

            parts = cpool.tile([128, 4], F32, name="parts")
            nc.vector.memset(parts[:], 0.0)
            nc.sync.dma_start(parts[0:1, 3:4], ins["ctrl8"][:])

            x0T_sb, W_sb, wf_sb = {}, {}, {}
            for g in TOWERS:
                x0T_sb[g] = cpool.tile([D, NB], BF16, name=f"x0T_sb_{g}")
                nc.sync.dma_start(x0T_sb[g][:], ins[f"x0T_{g}"][:])
                W_sb[g] = cpool.tile([128, L * D], BF16, name=f"W_sb_{g}")
                nc.sync.dma_start(W_sb[g].rearrange("p (l d) -> p l d", l=L), ins[f"W_{g}"].rearrange("l p d -> p l d"))
                wf_sb[g] = cpool.tile([128, L * 2], BF16, name=f"wf_sb_{g}")
                nc.sync.dma_start(wf_sb[g].rearrange("p (l j) -> p l j", l=L), ins[f"wf_{g}"].rearrange("l p j -> p l j"))

            def produce_block(g, l, xT_ap):
                """From xT [d, own-nodes] compute h-block (+s2 col) payload and
                s1 row for layer l; kick off the AllGather. Returns (s1row, cco)."""
                pay = work.tile([128, MC * 129], BF16, tag="pay", bufs=2,
                                name=f"pay_{g}{l}")
                for c in range(MC):
                    ph = psum.tile([128, 129], F32, tag="px", bufs=3,
                                   name=f"ph_{g}{l}{c}")
                    nc.tensor.matmul(ph[:, 0:128], xT_ap[:, c * 128:(c + 1) * 128],
                                     W_sb[g][:, l * D:(l + 1) * D],
                                     start=True, stop=True)
                    nc.tensor.matmul(ph[:, 128:129], xT_ap[:, c * 128:(c + 1) * 128],
                                     wf_sb[g][:, 2 * l + 1:2 * l + 2],
                                     start=True, stop=True)
                    nc.vector.tensor_copy(pay[:, c * 129:(c + 1) * 129], ph[:])
                ps1 = psum.tile([1, NB], F32, tag="pden4", bufs=3, name=f"ps1_{g}{l}")
                nc.tensor.matmul(ps1[:], wf_sb[g][:, 2 * l:2 * l + 1], xT_ap[:],
                                 start=True, stop=True)
                s1row = work.tile([1, NB], BF16, tag="s1row", bufs=3,
                                  name=f"s1row_{g}{l}")
                nc.vector.tensor_copy(s1row[:], ps1[:])

                cci = dpool.tile([MC, 128, 129], BF16, tag=f"cci_{g}", bufs=2,
                                 name=f"cci_{g}{l}")
                cco = dpool.tile([NCORES, MC, 128, 129], BF16, tag=f"cco_{g}",
                                 bufs=2, addr_space="Shared", name=f"cco_{g}{l}")
                nc.scalar.dma_start(cci.rearrange("c p j -> p c j"), pay.rearrange("p (c j) -> p c j", j=129))
                nc.gpsimd.collective_compute(
                    "AllGather", ALU.bypass, replica_groups=RG,
                    ins=[cci[:].opt()], outs=[cco[:].opt()])
                return s1row, cco

            NDVE = EGRP - NPRELU

            def chunkwork(g, l, s1row, cco, ti):
                """Softmax over all 4096 keys for this core's 512 queries of
                tower g, layer l, plus the attention matmul; returns new
                xT [d, own-nodes] (None on the last layer)."""
                last = (l == L - 1)
                h_sb = hpool.tile([128, KC * 129], BF16, tag="h", bufs=3,
                                  name=f"h_sb_{g}{l}")
                nc.sync.dma_start(h_sb.rearrange("p (g c j) -> p g c j", g=NCORES, c=MC), cco.rearrange("g c p j -> p g c j"))
                h3 = h_sb.rearrange("p (k j) -> p k j", j=129)
                s2f = work.tile([128, KC], F32, tag="s2f", bufs=2, name=f"s2f_{g}{l}")
                nc.vector.tensor_copy(s2f[:], h3[:, :, 128:129])
                s1d = dpool.tile([NB], BF16, tag="s1d", bufs=2, name=f"s1d_{g}{l}")
                nc.gpsimd.dma_start(s1d[:], s1row[:])
                s1b = work.tile([128, NB], BF16, tag="s1b", bufs=2, name=f"s1b_{g}{l}")
                nc.gpsimd.dma_start(s1b[:], s1d.rearrange("(a f) -> a f", a=1).broadcast_to([128, NB]))

                px = psum.tile([128, NB], F32, tag="px", bufs=3, name=f"px_{g}{l}")
                pden4 = psum.tile([128, NB], F32, tag="pden4", bufs=3,
                                  name=f"pden4_{g}{l}")
                for g0 in range(0, KC, EGRP):
                    wgrp = wpool.tile([128, EGRP * NB], BF16, tag="wgrp", bufs=3,
                                      name=f"wgrp_{g}{l}{g0}")
                    zgrp = wpool.tile([128, NDVE * NB], BF16, tag="zgrp", bufs=3,
                                      name=f"zgrp_{g}{l}{g0}")
                    for ko in range(NDVE):
                        kc = g0 + ko
                        nc.vector.tensor_scalar(zgrp[:, ko * NB:(ko + 1) * NB],
                                                s1b[:], s2f[:, kc:kc + 1],
                                                None, ALU.add)
                    nc.vector.scalar_tensor_tensor(wgrp[:, 0:NDVE * NB], zgrp[:],
                                                   0.01, zgrp[:], ALU.mult, ALU.max)
                    for ko in range(NDVE, EGRP):
                        kc = g0 + ko
                        nc.scalar.activation(wgrp[:, ko * NB:(ko + 1) * NB], s1b[:],
                                             AF.Prelu, bias=s2f[:, kc:kc + 1],
                                             scale=1.0, alpha=0.01)
                    egrp = wpool.tile([128, EGRP * NB], BF16, tag="egrp", bufs=4,
                                      name=f"egrp_{g}{l}{g0}")
                    nc.scalar.activation(egrp[:], wgrp[:], AF.Exp)
                    for ko in range(EGRP):
                        kc = g0 + ko
                        esl = slice(ko * NB, (ko + 1) * NB)
                        hsl = slice(kc * 129, kc * 129 + 128)
                        nc.tensor.matmul(px[:], h_sb[:, hsl], egrp[:, esl],
                                         start=(kc == 0), stop=(kc == KC - 1))
                        j = ko % 4
                        nc.tensor.matmul(pden4[32 * j:32 * j + 1, :], ones_col[:],
                                         egrp[:, esl], start=(kc < 4),
                                         stop=(kc >= KC - 4),
                                         tile_position=(0, 32 * j))

                # 1/den: bounce through DRAM into [128, 4] so reciprocal runs
                # 128 lanes wide. All hops on the idle gpsimd DMA queue to
                # keep this latency chain out of the busy bulk-DMA queues.
                denf = work.tile([128, NB], F32, tag="denf", bufs=2, name=f"denf_{g}{l}")
                nc.scalar.activation(denf[:], pden4[:], AF.Copy)
                dscr = dpool.tile([4, NB], F32, tag="dscr", bufs=2, name=f"dscr_{g}{l}")
                for j in range(4):
                    nc.gpsimd.dma_start(dscr[j], denf[32 * j:32 * j + 1, :])
                denc4 = work.tile([128, 16], F32, tag="denc4", bufs=2,
                                  name=f"denc4_{g}{l}")
                for j in range(4):
                    nc.gpsimd.dma_start(denc4.rearrange("p (c j) -> p c j", j=4)[:, :, j],
                                        dscr[j].rearrange("(c p) -> p c", p=128))
                den_c = work.tile([128, 4], F32, tag="den_c", bufs=2, name=f"denc_{g}{l}")
                nc.vector.tensor_reduce(den_c[:], denc4.rearrange("p (c j) -> p c j", j=4),
                                        AX.X, ALU.add)
                rec_c = work.tile([128, 4], F32, tag="rec_c", bufs=2, name=f"rec_{g}{l}")
                nc.vector.reciprocal(rec_c[:], den_c[:])
                dscr2 = dpool.tile([NB], F32, tag="dscr2", bufs=2, name=f"dscr2_{g}{l}")
                nc.gpsimd.dma_start(dscr2.rearrange("(c p) -> p c", p=128), rec_c[:])
                db = work.tile([128, NB], F32, tag="db", bufs=2, name=f"db_{g}{l}")
                nc.gpsimd.dma_start(db[:], dscr2.rearrange("(a f) -> a f", a=1).broadcast_to([128, NB]))
                xT = work.tile([128, NB], BF16, tag="xT", bufs=2, name=f"xT_{g}{l}")
                nc.vector.tensor_tensor(xT[:], px[:], db[:], ALU.mult)
                if last:
                    nc.vector.tensor_reduce(parts[:, ti:ti + 1], xT[:], AX.X, ALU.add)
                    return None
                return xT

            s1rows, ccos = {}, {}
            for g in TOWERS:
                s1rows[g], ccos[g] = produce_block(g, 0, x0T_sb[g])
            for l in range(L):
                for ti, g in enumerate(TOWERS):
                    xT = chunkwork(g, l, s1rows[g], ccos[g], ti)
                    if l < L - 1:
                        s1rows[g], ccos[g] = produce_block(g, l + 1, xT)

            # ---- tail: gather partials, reduce, replicated MLP head ----
            red_in = dpool.tile([128, 4], F32, name="red_in")
            red_out = dpool.tile([NCORES, 128, 4], F32, addr_space="Shared",
                                 name="red_out")
            nc.sync.dma_start(red_in[:], parts[:])
            nc.gpsimd.collective_compute(
                "AllGather", ALU.bypass, replica_groups=RG,
                ins=[red_in[:].opt()], outs=[red_out[:].opt()])
            red_sb = work.tile([128, NCORES * 4], F32, name="red_sb")
            for gg in range(NCORES):
                nc.sync.dma_start(red_sb[:, gg * 4:(gg + 1) * 4], red_out[gg])
            xcol = work.tile([128, 4], F32, name="xcol")
            nc.vector.tensor_reduce(xcol[:], red_sb.rearrange("p (g t) -> p t g", t=4),
                                    AX.X, ALU.add)

            wh0 = cpool.tile([128, 4, HP], F32, name="wh0")
            wh1 = cpool.tile([128, 4, HP], F32, name="wh1")
            wpj = cpool.tile([128, 4, D], F32, name="wpj")
            bh0 = cpool.tile([128, 4], F32, name="bh0s")
            bh1 = cpool.tile([128, 4], F32, name="bh1s")
            bpj = cpool.tile([128, 1], F32, name="bpjs")
            wo = cpool.tile([128, 1], F32, name="wos")
            bo = cpool.tile([1, 1], F32, name="bos")
            nc.sync.dma_start(wh0[:], ins["Wh0"].rearrange("(i p) c -> p i c", p=128))
            nc.sync.dma_start(wh1[:], ins["Wh1"].rearrange("(i p) c -> p i c", p=128))
            nc.sync.dma_start(wpj[:], ins["Wpj"].rearrange("(i p) c -> p i c", p=128))
            nc.sync.dma_start(bh0[:], ins["bh0"].rearrange("(j p) -> p j", p=128))
            nc.sync.dma_start(bh1[:], ins["bh1"].rearrange("(j p) -> p j", p=128))
            nc.sync.dma_start(bpj[:], ins["bpj"].rearrange("(j p) -> p j", p=128))
            nc.sync.dma_start(wo[:], ins["Wo"][:])
            nc.sync.dma_start(bo[:], ins["bo"].rearrange("(a b) -> a b", a=1))

            def head_layer(y_in, w_sb, b_sb, jo, name):
                y_out = work.tile([128, jo], F32, tag="ycol", bufs=2, name=name)
                ji = 4
                for j in range(jo):
                    py = psum.tile([128, 1], F32, tag="px", bufs=3,
                                   name=f"py_{name}{j}")
                    for i in range(ji):
                        nc.tensor.matmul(py[:], w_sb[:, i, j * 128:(j + 1) * 128],
                                         y_in[:, i:i + 1],
                                         start=(i == 0), stop=(i == ji - 1))
                    nc.scalar.activation(y_out[:, j:j + 1], py[:], AF.Gelu,
                                         bias=b_sb[:, j:j + 1])
                return y_out

            y1 = head_layer(xcol, wh0, bh0, 4, "y1")
            y2 = head_layer(y1, wh1, bh1, 4, "y2")
            y3 = head_layer(y2, wpj, bpj, 1, "y3")
            po = psum.tile([1, 1], F32, tag="pden4", bufs=3, name="po")
            nc.tensor.matmul(po[:], wo[:], y3[:], start=True, stop=True)
            o_sb = work.tile([1, 1], F32, name="o_sb")
            nc.scalar.activation(o_sb[:], po[:], AF.Identity, bias=bo[0:1, 0:1])
            nc.sync.dma_start(out_d[:], o_sb[:])

    nc.compile()
    return nc


def _prep_in_maps(inputs):
    bf16 = ml_dtypes.bfloat16
    f32 = np.float32

    def bf(a):
        return np.ascontiguousarray(np.asarray(a, dtype=f32)).astype(bf16)

    common = {}
    percore = [dict() for _ in range(NCORES)]
    for g in TOWERS:
        x0 = np.concatenate([np.asarray(inputs[f"{g}_a"], f32),
                             np.asarray(inputs[f"{g}_aim"], f32),
                             np.asarray(inputs[f"{g}_q"], f32)[:, None]], axis=1)
        x0T = np.ascontiguousarray(x0.T)          # [D, N]
        W = np.asarray(inputs[f"{g}_W"], f32)     # [L, D, D]
        att = np.asarray(inputs[f"{g}_att"], f32)  # [L, 2D]
        wf = np.stack([np.einsum("lij,lj->li", W, att[:, :D]),
                       np.einsum("lij,lj->li", W, att[:, D:])], axis=-1)  # [L,D,2]
        common[f"W_{g}"] = bf(W)
        common[f"wf_{g}"] = bf(wf)
        for c in range(NCORES):
            percore[c][f"x0T_{g}"] = bf(x0T[:, c * NB:(c + 1) * NB])

    common["ctrl8"] = np.asarray(inputs["control"], f32) / NCORES
    H = 3 * D + 1
    pad2 = lambda a, r, c: np.pad(np.asarray(a, f32), ((0, r - a.shape[0]), (0, c - a.shape[1])))
    pad1 = lambda a, r: np.pad(np.asarray(a, f32), (0, r - a.shape[0]))
    common["Wh0"] = pad2(inputs["W_h0"], HP, HP)
    common["bh0"] = pad1(inputs["b_h0"], HP)
    common["Wh1"] = pad2(inputs["W_h1"], HP, HP)
    common["bh1"] = pad1(inputs["b_h1"], HP)
    common["Wpj"] = pad2(inputs["W_proj"], HP, D)
    common["bpj"] = np.asarray(inputs["b_proj"], f32)
    common["Wo"] = np.asarray(inputs["W_out"], f32)
    common["bo"] = np.asarray(inputs["b_out"], f32)

    return [dict(common, **percore[c]) for c in range(NCORES)]


def kernel(**inputs):
    global LAST_RESULT, _CACHED
    from concourse.bass_utils import run_bass_kernel_spmd

    if _CACHED is None:
        _CACHED = _build()
    nc = _CACHED
    in_maps = _prep_in_maps(inputs)
    res = run_bass_kernel_spmd(nc, in_maps, core_ids=list(range(NCORES)))
    LAST_RESULT = res
    return np.asarray(res.results[0]["out"], np.float32)


# revision 12
# speedup vs baseline: 1.0362x; 1.0362x over previous
"""Trainium2 Bass kernel for nn_AmidePredictor (3x GAT towers + MLP head).

Strategy (8 NeuronCores, SPMD):
  - Each core owns NB=512 query nodes of every tower (row-sharded N x N attention).
  - Per layer: each core computes its h-block + score columns, AllGathers
    h(+s2) across the 8 cores, then does its 512-row slice of the
    exp(leaky(s1_i+s2_j)) softmax via DVE/ACT and accumulates
    numerator (h-weighted) + denominator on the TensorEngine in bf16.
  - The three towers are interleaved so each tower's AllGather overlaps the
    other towers' compute.
  - Node-sums (free-axis reduce in the transposed layout), one tiny final
    AllGather, then the replicated 385-wide MLP head on every core.

kernel(**inputs) takes the FULL inputs from setup_inputs() and returns the
FULL [1] output. Self-contained: only needs the concourse/bass runtime
(/opt/trn_rl_repo) and the 8 axon NeuronCores.
"""
import os
import sys

for _p in ("/opt/trn_rl_repo",):
    if _p not in sys.path and os.path.isdir(_p):
        sys.path.insert(0, _p)

import numpy as np
import ml_dtypes

N = 4096
D = 128
L = 3
NCORES = 8
NB = N // NCORES          # 512 own query nodes per core
KC = N // 128             # 32 key chunks of 128
MC = NB // 128            # 4 own-node chunks of 128
TOWERS = ("acid", "amine", "int")
HP = 512                  # padded head width (385 -> 512)

EGRP = 8                    # k-chunks per exp/wbig group tile
NPRELU = 2                  # chunks per group handled by ACT Prelu (rest: DVE)
NGRP = KC // EGRP           # groups per layer-tower

LAST_RESULT = None
_CACHED = None


def _build():
    import concourse.bass as bass
    import concourse.bacc as bacc
    import concourse.tile as tile
    import concourse.mybir as mybir

    F32 = mybir.dt.float32
    BF16 = mybir.dt.bfloat16
    AF = mybir.ActivationFunctionType
    ALU = mybir.AluOpType
    AX = mybir.AxisListType
    RG = [list(range(NCORES))]

    nc = bacc.Bacc("TRN2", target_bir_lowering=False, debug=False,
                   num_devices=NCORES)

    ins = {}
    for g in TOWERS:
        ins[f"x0T_{g}"] = nc.dram_tensor(f"x0T_{g}", [D, NB], BF16, kind="ExternalInput")
        ins[f"W_{g}"] = nc.dram_tensor(f"W_{g}", [D, L, D], BF16, kind="ExternalInput")
        ins[f"wf_{g}"] = nc.dram_tensor(f"wf_{g}", [D, L, 2], BF16, kind="ExternalInput")
    ins["ctrl8"] = nc.dram_tensor("ctrl8", [1], F32, kind="ExternalInput")
    ins["Wh0"] = nc.dram_tensor("Wh0", [128, 4, HP], F32, kind="ExternalInput")
    ins["bh0"] = nc.dram_tensor("bh0", [128, 4], F32, kind="ExternalInput")
    ins["Wh1"] = nc.dram_tensor("Wh1", [128, 4, HP], F32, kind="ExternalInput")
    ins["bh1"] = nc.dram_tensor("bh1", [128, 4], F32, kind="ExternalInput")
    ins["Wpj"] = nc.dram_tensor("Wpj", [128, 4, D], F32, kind="ExternalInput")
    ins["bpj"] = nc.dram_tensor("bpj", [128, 1], F32, kind="ExternalInput")
    ins["Wo"] = nc.dram_tensor("Wo", [D, 1], F32, kind="ExternalInput")
    ins["bo"] = nc.dram_tensor("bo", [1], F32, kind="ExternalInput")
    out_d = nc.dram_tensor("out", [1], F32, kind="ExternalOutput")

    with tile.TileContext(nc) as tc:
        with tc.tile_pool(name="cpool", bufs=1) as cpool, \
             tc.tile_pool(name="work", bufs=1) as work, \
             tc.tile_pool(name="hpool", bufs=1) as hpool, \
             tc.tile_pool(name="wpool", bufs=1) as wpool, \
             tc.tile_pool(name="psum", bufs=1, space=bass.MemorySpace.PSUM) as psum, \
             tc.tile_pool(name="dpool", bufs=1, space="DRAM") as dpool:

            ones_col = cpool.tile([128, 1], BF16, name="ones_col")
            nc.vector.memset(ones_col[:], 1.0)

            parts = cpool.tile([128, 4], F32, name="parts")
            nc.vector.memset(parts[:], 0.0)
            nc.sync.dma_start(parts[0:1, 3:4], ins["ctrl8"][:])

            x0T_sb, W_sb, wf_sb = {}, {}, {}
            for g in TOWERS:
                x0T_sb[g] = cpool.tile([D, NB], BF16, name=f"x0T_sb_{g}")
                nc.sync.dma_start(x0T_sb[g][:], ins[f"x0T_{g}"][:])
                W_sb[g] = cpool.tile([128, L * D], BF16, name=f"W_sb_{g}")
                nc.sync.dma_start(W_sb[g][:], ins[f"W_{g}"].rearrange("p l d -> p (l d)"))
                wf_sb[g] = cpool.tile([128, L * 2], BF16, name=f"wf_sb_{g}")
                nc.sync.dma_start(wf_sb[g][:], ins[f"wf_{g}"].rearrange("p l j -> p (l j)"))

            def produce_block(g, l, xT_ap):
                """From xT [d, own-nodes] compute h-block (+s2 col) payload and
                s1 row for layer l; kick off the AllGather. Returns (s1row, cco)."""
                pay = work.tile([128, MC * 129], BF16, tag="pay", bufs=2,
                                name=f"pay_{g}{l}")
                for c in range(MC):
                    ph = psum.tile([128, 129], F32, tag="px", bufs=3,
                                   name=f"ph_{g}{l}{c}")
                    nc.tensor.matmul(ph[:, 0:128], xT_ap[:, c * 128:(c + 1) * 128],
                                     W_sb[g][:, l * D:(l + 1) * D],
                                     start=True, stop=True)
                    nc.tensor.matmul(ph[:, 128:129], xT_ap[:, c * 128:(c + 1) * 128],
                                     wf_sb[g][:, 2 * l + 1:2 * l + 2],
                                     start=True, stop=True)
                    nc.vector.tensor_copy(pay[:, c * 129:(c + 1) * 129], ph[:])
                ps1 = psum.tile([1, NB], F32, tag="pden4", bufs=3, name=f"ps1_{g}{l}")
                nc.tensor.matmul(ps1[:], wf_sb[g][:, 2 * l:2 * l + 1], xT_ap[:],
                                 start=True, stop=True)
                s1row = work.tile([1, NB], BF16, tag="s1row", bufs=3,
                                  name=f"s1row_{g}{l}")
                nc.vector.tensor_copy(s1row[:], ps1[:])

                cci = dpool.tile([128, MC * 129], BF16, tag=f"cci_{g}", bufs=2,
                                 name=f"cci_{g}{l}")
                cco = dpool.tile([NCORES, 128, MC * 129], BF16, tag=f"cco_{g}",
                                 bufs=2, addr_space="Shared", name=f"cco_{g}{l}")
                nc.scalar.dma_start(cci[:], pay[:])
                nc.gpsimd.collective_compute(
                    "AllGather", ALU.bypass, replica_groups=RG,
                    ins=[cci[:].opt()], outs=[cco[:].opt()])
                return s1row, cco

            NDVE = EGRP - NPRELU

            def chunkwork(g, l, s1row, cco, ti):
                """Softmax over all 4096 keys for this core's 512 queries of
                tower g, layer l, plus the attention matmul; returns new
                xT [d, own-nodes] (None on the last layer)."""
                last = (l == L - 1)
                h_sb = hpool.tile([128, KC * 129], BF16, tag="h", bufs=3,
                                  name=f"h_sb_{g}{l}")
                nc.sync.dma_start(h_sb.rearrange("p (g x) -> p g x", g=NCORES), cco.rearrange("g p x -> p g x"))
                h3 = h_sb.rearrange("p (k j) -> p k j", j=129)  # k = 4*g + c order preserved
                s2f = work.tile([128, KC], F32, tag="s2f", bufs=2, name=f"s2f_{g}{l}")
                nc.vector.tensor_copy(s2f[:], h3[:, :, 128:129])
                s1d = dpool.tile([NB], BF16, tag="s1d", bufs=2, name=f"s1d_{g}{l}")
                nc.scalar.dma_start(s1d[:], s1row[:])
                s1b = work.tile([128, NB], BF16, tag="s1b", bufs=2, name=f"s1b_{g}{l}")
                nc.scalar.dma_start(s1b[:], s1d.rearrange("(a f) -> a f", a=1).broadcast_to([128, NB]))

                px = psum.tile([128, NB], F32, tag="px", bufs=3, name=f"px_{g}{l}")
                pden4 = psum.tile([128, NB], F32, tag="pden4", bufs=3,
                                  name=f"pden4_{g}{l}")
                for g0 in range(0, KC, EGRP):
                    wgrp = wpool.tile([128, EGRP * NB], BF16, tag="wgrp", bufs=3,
                                      name=f"wgrp_{g}{l}{g0}")
                    zgrp = wpool.tile([128, NDVE * NB], BF16, tag="zgrp", bufs=3,
                                      name=f"zgrp_{g}{l}{g0}")
                    for ko in range(NDVE):
                        kc = g0 + ko
                        nc.vector.tensor_scalar(zgrp[:, ko * NB:(ko + 1) * NB],
                                                s1b[:], s2f[:, kc:kc + 1],
                                                None, ALU.add)
                    nc.vector.scalar_tensor_tensor(wgrp[:, 0:NDVE * NB], zgrp[:],
                                                   0.01, zgrp[:], ALU.mult, ALU.max)
                    for ko in range(NDVE, EGRP):
                        kc = g0 + ko
                        nc.scalar.activation(wgrp[:, ko * NB:(ko + 1) * NB], s1b[:],
                                             AF.Prelu, bias=s2f[:, kc:kc + 1],
                                             scale=1.0, alpha=0.01)
                    egrp = wpool.tile([128, EGRP * NB], BF16, tag="egrp", bufs=4,
                                      name=f"egrp_{g}{l}{g0}")
                    nc.scalar.activation(egrp[:], wgrp[:], AF.Exp)
                    for ko in range(EGRP):
                        kc = g0 + ko
                        esl = slice(ko * NB, (ko + 1) * NB)
                        hsl = slice(kc * 129, kc * 129 + 128)
                        nc.tensor.matmul(px[:], h_sb[:, hsl], egrp[:, esl],
                                         start=(kc == 0), stop=(kc == KC - 1))
                        j = ko % 4
                        nc.tensor.matmul(pden4[32 * j:32 * j + 1, :], ones_col[:],
                                         egrp[:, esl], start=(kc < 4),
                                         stop=(kc >= KC - 4),
                                         tile_position=(0, 32 * j))

                # 1/den: bounce through DRAM into [128, 4] so reciprocal runs
                # 128 lanes wide. All hops on the idle gpsimd DMA queue to
                # keep this latency chain out of the busy bulk-DMA queues.
                denf = work.tile([128, NB], F32, tag="denf", bufs=2, name=f"denf_{g}{l}")
                nc.scalar.activation(denf[:], pden4[:], AF.Copy)
                dscr = dpool.tile([4, NB], F32, tag="dscr", bufs=2, name=f"dscr_{g}{l}")
                for j in range(4):
                    nc.scalar.dma_start(dscr[j], denf[32 * j:32 * j + 1, :])
                denc4 = work.tile([128, 16], F32, tag="denc4", bufs=2,
                                  name=f"denc4_{g}{l}")
                for j in range(4):
                    nc.scalar.dma_start(denc4.rearrange("p (c j) -> p c j", j=4)[:, :, j],
                                        dscr[j].rearrange("(c p) -> p c", p=128))
                den_c = work.tile([128, 4], F32, tag="den_c", bufs=2, name=f"denc_{g}{l}")
                nc.vector.tensor_reduce(den_c[:], denc4.rearrange("p (c j) -> p c j", j=4),
                                        AX.X, ALU.add)
                rec_c = work.tile([128, 4], F32, tag="rec_c", bufs=2, name=f"rec_{g}{l}")
                nc.vector.reciprocal(rec_c[:], den_c[:])
                dscr2 = dpool.tile([NB], F32, tag="dscr2", bufs=2, name=f"dscr2_{g}{l}")
                nc.scalar.dma_start(dscr2.rearrange("(c p) -> p c", p=128), rec_c[:])
                db = work.tile([128, NB], F32, tag="db", bufs=2, name=f"db_{g}{l}")
                nc.scalar.dma_start(db[:], dscr2.rearrange("(a f) -> a f", a=1).broadcast_to([128, NB]))
                xT = work.tile([128, NB], BF16, tag="xT", bufs=2, name=f"xT_{g}{l}")
                nc.vector.tensor_tensor(xT[:], px[:], db[:], ALU.mult)
                if last:
                    nc.vector.tensor_reduce(parts[:, ti:ti + 1], xT[:], AX.X, ALU.add)
                    return None
                return xT

            s1rows, ccos = {}, {}
            for g in TOWERS:
                s1rows[g], ccos[g] = produce_block(g, 0, x0T_sb[g])
            for l in range(L):
                for ti, g in enumerate(TOWERS):
                    xT = chunkwork(g, l, s1rows[g], ccos[g], ti)
                    if l < L - 1:
                        s1rows[g], ccos[g] = produce_block(g, l + 1, xT)

            # ---- tail: gather partials, reduce, replicated MLP head ----
            red_in = dpool.tile([128, 4], F32, name="red_in")
            red_out = dpool.tile([NCORES, 128, 4], F32, addr_space="Shared",
                                 name="red_out")
            nc.sync.dma_start(red_in[:], parts[:])
            nc.gpsimd.collective_compute(
                "AllGather", ALU.bypass, replica_groups=RG,
                ins=[red_in[:].opt()], outs=[red_out[:].opt()])
            red_sb = work.tile([128, NCORES * 4], F32, name="red_sb")
            for gg in range(NCORES):
                nc.sync.dma_start(red_sb[:, gg * 4:(gg + 1) * 4], red_out[gg])
            xcol = work.tile([128, 4], F32, name="xcol")
            nc.vector.tensor_reduce(xcol[:], red_sb.rearrange("p (g t) -> p t g", t=4),
                                    AX.X, ALU.add)

            wh0 = cpool.tile([128, 4, HP], F32, name="wh0")
            wh1 = cpool.tile([128, 4, HP], F32, name="wh1")
            wpj = cpool.tile([128, 4, D], F32, name="wpj")
            bh0 = cpool.tile([128, 4], F32, name="bh0s")
            bh1 = cpool.tile([128, 4], F32, name="bh1s")
            bpj = cpool.tile([128, 1], F32, name="bpjs")
            wo = cpool.tile([128, 1], F32, name="wos")
            bo = cpool.tile([1, 1], F32, name="bos")
            nc.sync.dma_start(wh0[:], ins["Wh0"][:])
            nc.sync.dma_start(wh1[:], ins["Wh1"][:])
            nc.sync.dma_start(wpj[:], ins["Wpj"][:])
            nc.sync.dma_start(bh0[:], ins["bh0"][:])
            nc.sync.dma_start(bh1[:], ins["bh1"][:])
            nc.sync.dma_start(bpj[:], ins["bpj"][:])
            nc.sync.dma_start(wo[:], ins["Wo"][:])
            nc.sync.dma_start(bo[:], ins["bo"].rearrange("(a b) -> a b", a=1))

            def head_layer(y_in, w_sb, b_sb, jo, name):
                y_out = work.tile([128, jo], F32, tag="ycol", bufs=2, name=name)
                ji = 4
                for j in range(jo):
                    py = psum.tile([128, 1], F32, tag="px", bufs=3,
                                   name=f"py_{name}{j}")
                    for i in range(ji):
                        nc.tensor.matmul(py[:], w_sb[:, i, j * 128:(j + 1) * 128],
                                         y_in[:, i:i + 1],
                                         start=(i == 0), stop=(i == ji - 1))
                    nc.scalar.activation(y_out[:, j:j + 1], py[:], AF.Gelu,
                                         bias=b_sb[:, j:j + 1])
                return y_out

            y1 = head_layer(xcol, wh0, bh0, 4, "y1")
            y2 = head_layer(y1, wh1, bh1, 4, "y2")
            y3 = head_layer(y2, wpj, bpj, 1, "y3")
            po = psum.tile([1, 1], F32, tag="pden4", bufs=3, name="po")
            nc.tensor.matmul(po[:], wo[:], y3[:], start=True, stop=True)
            o_sb = work.tile([1, 1], F32, name="o_sb")
            nc.scalar.activation(o_sb[:], po[:], AF.Identity, bias=bo[0:1, 0:1])
            nc.sync.dma_start(out_d[:], o_sb[:])

    nc.compile()
    return nc


def _prep_in_maps(inputs):
    bf16 = ml_dtypes.bfloat16
    f32 = np.float32

    def bf(a):
        return np.ascontiguousarray(np.asarray(a, dtype=f32)).astype(bf16)

    common = {}
    percore = [dict() for _ in range(NCORES)]
    for g in TOWERS:
        x0 = np.concatenate([np.asarray(inputs[f"{g}_a"], f32),
                             np.asarray(inputs[f"{g}_aim"], f32),
                             np.asarray(inputs[f"{g}_q"], f32)[:, None]], axis=1)
        x0T = np.ascontiguousarray(x0.T)          # [D, N]
        W = np.asarray(inputs[f"{g}_W"], f32)     # [L, D, D]
        att = np.asarray(inputs[f"{g}_att"], f32)  # [L, 2D]
        wf = np.stack([np.einsum("lij,lj->li", W, att[:, :D]),
                       np.einsum("lij,lj->li", W, att[:, D:])], axis=-1)  # [L,D,2]
        common[f"W_{g}"] = bf(W.transpose(1, 0, 2))        # [D(in), L, D(out)]
        common[f"wf_{g}"] = bf(wf.transpose(1, 0, 2))      # [D(in), L, 2]
        for c in range(NCORES):
            percore[c][f"x0T_{g}"] = bf(x0T[:, c * NB:(c + 1) * NB])

    common["ctrl8"] = np.asarray(inputs["control"], f32) / NCORES
    H = 3 * D + 1
    pad2 = lambda a, r, c: np.pad(np.asarray(a, f32), ((0, r - a.shape[0]), (0, c - a.shape[1])))
    pad1 = lambda a, r: np.pad(np.asarray(a, f32), (0, r - a.shape[0]))
    ptile = lambda a: np.ascontiguousarray(a.reshape(4, 128, -1).transpose(1, 0, 2))
    common["Wh0"] = ptile(pad2(inputs["W_h0"], HP, HP))      # [128, 4, HP]
    common["bh0"] = np.ascontiguousarray(pad1(inputs["b_h0"], HP).reshape(4, 128).T)
    common["Wh1"] = ptile(pad2(inputs["W_h1"], HP, HP))
    common["bh1"] = np.ascontiguousarray(pad1(inputs["b_h1"], HP).reshape(4, 128).T)
    common["Wpj"] = ptile(pad2(inputs["W_proj"], HP, D))     # [128, 4, D]
    common["bpj"] = np.asarray(inputs["b_proj"], f32).reshape(128, 1)
    common["Wo"] = np.asarray(inputs["W_out"], f32)
    common["bo"] = np.asarray(inputs["b_out"], f32)

    return [dict(common, **percore[c]) for c in range(NCORES)]


def kernel(**inputs):
    global LAST_RESULT, _CACHED
    from concourse.bass_utils import run_bass_kernel_spmd

    if _CACHED is None:
        _CACHED = _build()
    nc = _CACHED
    in_maps = _prep_in_maps(inputs)
    res = run_bass_kernel_spmd(nc, in_maps, core_ids=list(range(NCORES)))
    LAST_RESULT = res
    return np.asarray(res.results[0]["out"], np.float32)


# revision 13
# speedup vs baseline: 1.0471x; 1.0105x over previous
"""Trainium2 Bass kernel for nn_AmidePredictor (3x GAT towers + MLP head).

Strategy (8 NeuronCores, SPMD):
  - Each core owns NB=512 query nodes of every tower (row-sharded N x N attention).
  - Per layer: each core computes its h-block + score columns, AllGathers
    h(+s2) across the 8 cores, then does its 512-row slice of the
    exp(leaky(s1_i+s2_j)) softmax via DVE/ACT and accumulates
    numerator (h-weighted) + denominator on the TensorEngine in bf16.
  - The three towers are interleaved so each tower's AllGather overlaps the
    other towers' compute.
  - Node-sums (free-axis reduce in the transposed layout), one tiny final
    AllGather, then the replicated 385-wide MLP head on every core.

kernel(**inputs) takes the FULL inputs from setup_inputs() and returns the
FULL [1] output. Self-contained: only needs the concourse/bass runtime
(/opt/trn_rl_repo) and the 8 axon NeuronCores.
"""
import os
import sys

for _p in ("/opt/trn_rl_repo",):
    if _p not in sys.path and os.path.isdir(_p):
        sys.path.insert(0, _p)

import numpy as np
import ml_dtypes

N = 4096
D = 128
L = 3
NCORES = 8
NB = N // NCORES          # 512 own query nodes per core
KC = N // 128             # 32 key chunks of 128
MC = NB // 128            # 4 own-node chunks of 128
TOWERS = ("acid", "amine", "int")
HP = 512                  # padded head width (385 -> 512)

EGRP = 8                    # k-chunks per exp/wbig group tile
NPRELU = 2                  # chunks per group handled by ACT Prelu (rest: DVE)
NGRP = KC // EGRP           # groups per layer-tower

LAST_RESULT = None
_CACHED = None


def _build():
    import concourse.bass as bass
    import concourse.bacc as bacc
    import concourse.tile as tile
    import concourse.mybir as mybir

    F32 = mybir.dt.float32
    BF16 = mybir.dt.bfloat16
    AF = mybir.ActivationFunctionType
    ALU = mybir.AluOpType
    AX = mybir.AxisListType
    RG = [list(range(NCORES))]

    nc = bacc.Bacc("TRN2", target_bir_lowering=False, debug=False,
                   num_devices=NCORES)

    ins = {}
    for g in TOWERS:
        ins[f"x0T_{g}"] = nc.dram_tensor(f"x0T_{g}", [D, NB], BF16, kind="ExternalInput")
        ins[f"W_{g}"] = nc.dram_tensor(f"W_{g}", [D, L, D], BF16, kind="ExternalInput")
        ins[f"wf_{g}"] = nc.dram_tensor(f"wf_{g}", [D, L, 2], BF16, kind="ExternalInput")
    ins["ctrl8"] = nc.dram_tensor("ctrl8", [1], F32, kind="ExternalInput")
    ins["Wh0"] = nc.dram_tensor("Wh0", [128, 4, HP], F32, kind="ExternalInput")
    ins["bh0"] = nc.dram_tensor("bh0", [128, 4], F32, kind="ExternalInput")
    ins["Wh1"] = nc.dram_tensor("Wh1", [128, 4, HP], F32, kind="ExternalInput")
    ins["bh1"] = nc.dram_tensor("bh1", [128, 4], F32, kind="ExternalInput")
    ins["Wpj"] = nc.dram_tensor("Wpj", [128, 4, D], F32, kind="ExternalInput")
    ins["bpj"] = nc.dram_tensor("bpj", [128, 1], F32, kind="ExternalInput")
    ins["Wo"] = nc.dram_tensor("Wo", [D, 1], F32, kind="ExternalInput")
    ins["bo"] = nc.dram_tensor("bo", [1], F32, kind="ExternalInput")
    out_d = nc.dram_tensor("out", [1], F32, kind="ExternalOutput")

    with tile.TileContext(nc) as tc:
        with tc.tile_pool(name="cpool", bufs=1) as cpool, \
             tc.tile_pool(name="work", bufs=1) as work, \
             tc.tile_pool(name="hpool", bufs=1) as hpool, \
             tc.tile_pool(name="wpool", bufs=1) as wpool, \
             tc.tile_pool(name="psum", bufs=1, space=bass.MemorySpace.PSUM) as psum, \
             tc.tile_pool(name="dpool", bufs=1, space="DRAM") as dpool:

            ones_col = cpool.tile([128, 1], BF16, name="ones_col")
            nc.vector.memset(ones_col[:], 1.0)

            parts = cpool.tile([128, 4], F32, name="parts")
            nc.vector.memset(parts[:], 0.0)
            nc.sync.dma_start(parts[0:1, 3:4], ins["ctrl8"][:])

            x0T_sb, W_sb, wf_sb = {}, {}, {}
            for g in TOWERS:
                x0T_sb[g] = cpool.tile([D, NB], BF16, name=f"x0T_sb_{g}")
                nc.sync.dma_start(x0T_sb[g][:], ins[f"x0T_{g}"][:])
                W_sb[g] = cpool.tile([128, L * D], BF16, name=f"W_sb_{g}")
                nc.sync.dma_start(W_sb[g][:], ins[f"W_{g}"].rearrange("p l d -> p (l d)"))
                wf_sb[g] = cpool.tile([128, L * 2], BF16, name=f"wf_sb_{g}")
                nc.sync.dma_start(wf_sb[g][:], ins[f"wf_{g}"].rearrange("p l j -> p (l j)"))

            def produce_block(g, l, xT_ap):
                """From xT [d, own-nodes] compute h-block (+s2 col) payload and
                s1 row for layer l; kick off the AllGather. Returns (s1row, cco)."""
                pay = work.tile([128, MC * 129], BF16, tag="pay", bufs=2,
                                name=f"pay_{g}{l}")
                for c in range(MC):
                    ph = psum.tile([128, 129], F32, tag="px", bufs=3,
                                   name=f"ph_{g}{l}{c}")
                    nc.tensor.matmul(ph[:, 0:128], xT_ap[:, c * 128:(c + 1) * 128],
                                     W_sb[g][:, l * D:(l + 1) * D],
                                     start=True, stop=True)
                    nc.tensor.matmul(ph[:, 128:129], xT_ap[:, c * 128:(c + 1) * 128],
                                     wf_sb[g][:, 2 * l + 1:2 * l + 2],
                                     start=True, stop=True)
                    nc.vector.tensor_copy(pay[:, c * 129:(c + 1) * 129], ph[:])
                ps1 = psum.tile([1, NB], F32, tag="pden4", bufs=3, name=f"ps1_{g}{l}")
                nc.tensor.matmul(ps1[:], wf_sb[g][:, 2 * l:2 * l + 1], xT_ap[:],
                                 start=True, stop=True)
                s1row = work.tile([1, NB], BF16, tag="s1row", bufs=3,
                                  name=f"s1row_{g}{l}")
                nc.vector.tensor_copy(s1row[:], ps1[:])

                cci = dpool.tile([128, MC * 129], BF16, tag=f"cci_{g}", bufs=2,
                                 name=f"cci_{g}{l}")
                cco = dpool.tile([NCORES, 128, MC * 129], BF16, tag=f"cco_{g}",
                                 bufs=2, addr_space="Shared", name=f"cco_{g}{l}")
                nc.sync.dma_start(cci[:], pay[:])
                nc.gpsimd.collective_compute(
                    "AllGather", ALU.bypass, replica_groups=RG,
                    ins=[cci[:].opt()], outs=[cco[:].opt()])
                return s1row, cco

            NDVE = EGRP - NPRELU

            def chunkwork(g, l, s1row, cco, ti):
                """Softmax over all 4096 keys for this core's 512 queries of
                tower g, layer l, plus the attention matmul; returns new
                xT [d, own-nodes] (None on the last layer)."""
                last = (l == L - 1)
                h_sb = hpool.tile([128, KC * 129], BF16, tag="h", bufs=3,
                                  name=f"h_sb_{g}{l}")
                nc.sync.dma_start(h_sb.rearrange("p (g x) -> p g x", g=NCORES), cco.rearrange("g p x -> p g x"))
                h3 = h_sb.rearrange("p (k j) -> p k j", j=129)  # k = 4*g + c order preserved
                s2f = work.tile([128, KC], F32, tag="s2f", bufs=2, name=f"s2f_{g}{l}")
                nc.vector.tensor_copy(s2f[:], h3[:, :, 128:129])
                s1d = dpool.tile([NB], BF16, tag="s1d", bufs=2, name=f"s1d_{g}{l}")
                nc.sync.dma_start(s1d[:], s1row[:])
                s1b = work.tile([128, NB], BF16, tag="s1b", bufs=2, name=f"s1b_{g}{l}")
                nc.sync.dma_start(s1b[:], s1d.rearrange("(a f) -> a f", a=1).broadcast_to([128, NB]))

                px = psum.tile([128, NB], F32, tag="px", bufs=3, name=f"px_{g}{l}")
                pden4 = psum.tile([128, NB], F32, tag="pden4", bufs=3,
                                  name=f"pden4_{g}{l}")
                for g0 in range(0, KC, EGRP):
                    wgrp = wpool.tile([128, EGRP * NB], BF16, tag="wgrp", bufs=3,
                                      name=f"wgrp_{g}{l}{g0}")
                    zgrp = wpool.tile([128, NDVE * NB], BF16, tag="zgrp", bufs=3,
                                      name=f"zgrp_{g}{l}{g0}")
                    for ko in range(NDVE):
                        kc = g0 + ko
                        nc.vector.tensor_scalar(zgrp[:, ko * NB:(ko + 1) * NB],
                                                s1b[:], s2f[:, kc:kc + 1],
                                                None, ALU.add)
                    nc.vector.scalar_tensor_tensor(wgrp[:, 0:NDVE * NB], zgrp[:],
                                                   0.01, zgrp[:], ALU.mult, ALU.max)
                    for ko in range(NDVE, EGRP):
                        kc = g0 + ko
                        nc.scalar.activation(wgrp[:, ko * NB:(ko + 1) * NB], s1b[:],
                                             AF.Prelu, bias=s2f[:, kc:kc + 1],
                                             scale=1.0, alpha=0.01)
                    egrp = wpool.tile([128, EGRP * NB], BF16, tag="egrp", bufs=4,
                                      name=f"egrp_{g}{l}{g0}")
                    nc.scalar.activation(egrp[:], wgrp[:], AF.Exp)
                    for ko in range(EGRP):
                        kc = g0 + ko
                        esl = slice(ko * NB, (ko + 1) * NB)
                        hsl = slice(kc * 129, kc * 129 + 128)
                        nc.tensor.matmul(px[:], h_sb[:, hsl], egrp[:, esl],
                                         start=(kc == 0), stop=(kc == KC - 1))
                        j = ko % 4
                        nc.tensor.matmul(pden4[32 * j:32 * j + 1, :], ones_col[:],
                                         egrp[:, esl], start=(kc < 4),
                                         stop=(kc >= KC - 4),
                                         tile_position=(0, 32 * j))

                # 1/den: bounce through DRAM into [128, 4] so reciprocal runs
                # 128 lanes wide. All hops on the idle gpsimd DMA queue to
                # keep this latency chain out of the busy bulk-DMA queues.
                denf = work.tile([128, NB], F32, tag="denf", bufs=2, name=f"denf_{g}{l}")
                nc.scalar.activation(denf[:], pden4[:], AF.Copy)
                dscr = dpool.tile([4, NB], F32, tag="dscr", bufs=2, name=f"dscr_{g}{l}")
                for j in range(4):
                    nc.gpsimd.dma_start(dscr[j], denf[32 * j:32 * j + 1, :])
                denc4 = work.tile([128, 16], F32, tag="denc4", bufs=2,
                                  name=f"denc4_{g}{l}")
                for j in range(4):
                    nc.gpsimd.dma_start(denc4.rearrange("p (c j) -> p c j", j=4)[:, :, j],
                                        dscr[j].rearrange("(c p) -> p c", p=128))
                den_c = work.tile([128, 4], F32, tag="den_c", bufs=2, name=f"denc_{g}{l}")
                nc.vector.tensor_reduce(den_c[:], denc4.rearrange("p (c j) -> p c j", j=4),
                                        AX.X, ALU.add)
                rec_c = work.tile([128, 4], F32, tag="rec_c", bufs=2, name=f"rec_{g}{l}")
                nc.vector.reciprocal(rec_c[:], den_c[:])
                dscr2 = dpool.tile([NB], F32, tag="dscr2", bufs=2, name=f"dscr2_{g}{l}")
                nc.gpsimd.dma_start(dscr2.rearrange("(c p) -> p c", p=128), rec_c[:])
                db = work.tile([128, NB], F32, tag="db", bufs=2, name=f"db_{g}{l}")
                nc.sync.dma_start(db[:], dscr2.rearrange("(a f) -> a f", a=1).broadcast_to([128, NB]))
                xT = work.tile([128, NB], BF16, tag="xT", bufs=2, name=f"xT_{g}{l}")
                nc.vector.tensor_tensor(xT[:], px[:], db[:], ALU.mult)
                if last:
                    nc.vector.tensor_reduce(parts[:, ti:ti + 1], xT[:], AX.X, ALU.add)
                    return None
                return xT

            s1rows, ccos = {}, {}
            for g in TOWERS:
                s1rows[g], ccos[g] = produce_block(g, 0, x0T_sb[g])
            for l in range(L):
                for ti, g in enumerate(TOWERS):
                    xT = chunkwork(g, l, s1rows[g], ccos[g], ti)
                    if l < L - 1:
                        s1rows[g], ccos[g] = produce_block(g, l + 1, xT)

            # ---- tail: gather partials, reduce, replicated MLP head ----
            red_in = dpool.tile([128, 4], F32, name="red_in")
            red_out = dpool.tile([NCORES, 128, 4], F32, addr_space="Shared",
                                 name="red_out")
            nc.sync.dma_start(red_in[:], parts[:])
            nc.gpsimd.collective_compute(
                "AllGather", ALU.bypass, replica_groups=RG,
                ins=[red_in[:].opt()], outs=[red_out[:].opt()])
            red_sb = work.tile([128, NCORES * 4], F32, name="red_sb")
            for gg in range(NCORES):
                nc.sync.dma_start(red_sb[:, gg * 4:(gg + 1) * 4], red_out[gg])
            xcol = work.tile([128, 4], F32, name="xcol")
            nc.vector.tensor_reduce(xcol[:], red_sb.rearrange("p (g t) -> p t g", t=4),
                                    AX.X, ALU.add)

            wh0 = cpool.tile([128, 4, HP], F32, name="wh0")
            wh1 = cpool.tile([128, 4, HP], F32, name="wh1")
            wpj = cpool.tile([128, 4, D], F32, name="wpj")
            bh0 = cpool.tile([128, 4], F32, name="bh0s")
            bh1 = cpool.tile([128, 4], F32, name="bh1s")
            bpj = cpool.tile([128, 1], F32, name="bpjs")
            wo = cpool.tile([128, 1], F32, name="wos")
            bo = cpool.tile([1, 1], F32, name="bos")
            nc.sync.dma_start(wh0[:], ins["Wh0"][:])
            nc.sync.dma_start(wh1[:], ins["Wh1"][:])
            nc.sync.dma_start(wpj[:], ins["Wpj"][:])
            nc.sync.dma_start(bh0[:], ins["bh0"][:])
            nc.sync.dma_start(bh1[:], ins["bh1"][:])
            nc.sync.dma_start(bpj[:], ins["bpj"][:])
            nc.sync.dma_start(wo[:], ins["Wo"][:])
            nc.sync.dma_start(bo[:], ins["bo"].rearrange("(a b) -> a b", a=1))

            def head_layer(y_in, w_sb, b_sb, jo, name):
                y_out = work.tile([128, jo], F32, tag="ycol", bufs=2, name=name)
                ji = 4
                for j in range(jo):
                    py = psum.tile([128, 1], F32, tag="px", bufs=3,
                                   name=f"py_{name}{j}")
                    for i in range(ji):
                        nc.tensor.matmul(py[:], w_sb[:, i, j * 128:(j + 1) * 128],
                                         y_in[:, i:i + 1],
                                         start=(i == 0), stop=(i == ji - 1))
                    nc.scalar.activation(y_out[:, j:j + 1], py[:], AF.Gelu,
                                         bias=b_sb[:, j:j + 1])
                return y_out

            y1 = head_layer(xcol, wh0, bh0, 4, "y1")
            y2 = head_layer(y1, wh1, bh1, 4, "y2")
            y3 = head_layer(y2, wpj, bpj, 1, "y3")
            po = psum.tile([1, 1], F32, tag="pden4", bufs=3, name="po")
            nc.tensor.matmul(po[:], wo[:], y3[:], start=True, stop=True)
            o_sb = work.tile([1, 1], F32, name="o_sb")
            nc.scalar.activation(o_sb[:], po[:], AF.Identity, bias=bo[0:1, 0:1])
            nc.sync.dma_start(out_d[:], o_sb[:])

    nc.compile()
    return nc


def _prep_in_maps(inputs):
    bf16 = ml_dtypes.bfloat16
    f32 = np.float32

    def bf(a):
        return np.ascontiguousarray(np.asarray(a, dtype=f32)).astype(bf16)

    common = {}
    percore = [dict() for _ in range(NCORES)]
    for g in TOWERS:
        x0 = np.concatenate([np.asarray(inputs[f"{g}_a"], f32),
                             np.asarray(inputs[f"{g}_aim"], f32),
                             np.asarray(inputs[f"{g}_q"], f32)[:, None]], axis=1)
        x0T = np.ascontiguousarray(x0.T)          # [D, N]
        W = np.asarray(inputs[f"{g}_W"], f32)     # [L, D, D]
        att = np.asarray(inputs[f"{g}_att"], f32)  # [L, 2D]
        wf = np.stack([np.einsum("lij,lj->li", W, att[:, :D]),
                       np.einsum("lij,lj->li", W, att[:, D:])], axis=-1)  # [L,D,2]
        common[f"W_{g}"] = bf(W.transpose(1, 0, 2))        # [D(in), L, D(out)]
        common[f"wf_{g}"] = bf(wf.transpose(1, 0, 2))      # [D(in), L, 2]
        for c in range(NCORES):
            percore[c][f"x0T_{g}"] = bf(x0T[:, c * NB:(c + 1) * NB])

    common["ctrl8"] = np.asarray(inputs["control"], f32) / NCORES
    H = 3 * D + 1
    pad2 = lambda a, r, c: np.pad(np.asarray(a, f32), ((0, r - a.shape[0]), (0, c - a.shape[1])))
    pad1 = lambda a, r: np.pad(np.asarray(a, f32), (0, r - a.shape[0]))
    ptile = lambda a: np.ascontiguousarray(a.reshape(4, 128, -1).transpose(1, 0, 2))
    common["Wh0"] = ptile(pad2(inputs["W_h0"], HP, HP))      # [128, 4, HP]
    common["bh0"] = np.ascontiguousarray(pad1(inputs["b_h0"], HP).reshape(4, 128).T)
    common["Wh1"] = ptile(pad2(inputs["W_h1"], HP, HP))
    common["bh1"] = np.ascontiguousarray(pad1(inputs["b_h1"], HP).reshape(4, 128).T)
    common["Wpj"] = ptile(pad2(inputs["W_proj"], HP, D))     # [128, 4, D]
    common["bpj"] = np.asarray(inputs["b_proj"], f32).reshape(128, 1)
    common["Wo"] = np.asarray(inputs["W_out"], f32)
    common["bo"] = np.asarray(inputs["b_out"], f32)

    return [dict(common, **percore[c]) for c in range(NCORES)]


def kernel(**inputs):
    global LAST_RESULT, _CACHED
    from concourse.bass_utils import run_bass_kernel_spmd

    if _CACHED is None:
        _CACHED = _build()
    nc = _CACHED
    in_maps = _prep_in_maps(inputs)
    res = run_bass_kernel_spmd(nc, in_maps, core_ids=list(range(NCORES)))
    LAST_RESULT = res
    return np.asarray(res.results[0]["out"], np.float32)


# revision 14
# speedup vs baseline: 1.2178x; 1.1631x over previous
"""Trainium2 Bass kernel for nn_AmidePredictor (3x GAT towers + MLP head).

Strategy (8 NeuronCores, SPMD):
  - Each core owns NB=512 query nodes of every tower (row-sharded N x N attention).
  - Per layer: each core computes its h-block + score columns, AllGathers
    h(+s2) across the 8 cores, then does its 512-row slice of the
    exp(leaky(s1_i+s2_j)) softmax via DVE/ACT and accumulates
    numerator (h-weighted) + denominator on the TensorEngine in bf16.
  - The three towers are interleaved so each tower's AllGather overlaps the
    other towers' compute.
  - Node-sums (free-axis reduce in the transposed layout), one tiny final
    AllGather, then the replicated 385-wide MLP head on every core.

kernel(**inputs) takes the FULL inputs from setup_inputs() and returns the
FULL [1] output. Self-contained: only needs the concourse/bass runtime
(/opt/trn_rl_repo) and the 8 axon NeuronCores.
"""
import os
import sys

for _p in ("/opt/trn_rl_repo",):
    if _p not in sys.path and os.path.isdir(_p):
        sys.path.insert(0, _p)

import numpy as np
import ml_dtypes

N = 4096
D = 128
L = 3
NCORES = 8
NB = N // NCORES          # 512 own query nodes per core
KC = N // 128             # 32 key chunks of 128
MC = NB // 128            # 4 own-node chunks of 128
TOWERS = ("acid", "amine", "int")
HP = 512                  # padded head width (385 -> 512)

EGRP = 8                    # k-chunks per exp/wbig group tile
NPRELU = 2                  # chunks per group handled by ACT Prelu (rest: DVE)
NGRP = KC // EGRP           # groups per layer-tower

LAST_RESULT = None
_CACHED = None


def _build():
    import concourse.bass as bass
    import concourse.bacc as bacc
    import concourse.tile as tile
    import concourse.mybir as mybir

    F32 = mybir.dt.float32
    BF16 = mybir.dt.bfloat16
    AF = mybir.ActivationFunctionType
    ALU = mybir.AluOpType
    AX = mybir.AxisListType
    RG = [list(range(NCORES))]

    nc = bacc.Bacc("TRN2", target_bir_lowering=False, debug=False,
                   num_devices=NCORES)

    ins = {}
    for g in TOWERS:
        ins[f"x0T_{g}"] = nc.dram_tensor(f"x0T_{g}", [D, NB], BF16, kind="ExternalInput")
        ins[f"W_{g}"] = nc.dram_tensor(f"W_{g}", [D, L, D], BF16, kind="ExternalInput")
        ins[f"wf_{g}"] = nc.dram_tensor(f"wf_{g}", [D, L, 2], BF16, kind="ExternalInput")
    ins["ctrl8"] = nc.dram_tensor("ctrl8", [1], F32, kind="ExternalInput")
    ins["Wh0"] = nc.dram_tensor("Wh0", [128, 4, HP], F32, kind="ExternalInput")
    ins["bh0"] = nc.dram_tensor("bh0", [128, 4], F32, kind="ExternalInput")
    ins["Wh1"] = nc.dram_tensor("Wh1", [128, 4, HP], F32, kind="ExternalInput")
    ins["bh1"] = nc.dram_tensor("bh1", [128, 4], F32, kind="ExternalInput")
    ins["Wpj"] = nc.dram_tensor("Wpj", [128, 4, D], F32, kind="ExternalInput")
    ins["bpj"] = nc.dram_tensor("bpj", [128, 1], F32, kind="ExternalInput")
    ins["Wo"] = nc.dram_tensor("Wo", [D, 1], F32, kind="ExternalInput")
    ins["bo"] = nc.dram_tensor("bo", [1], F32, kind="ExternalInput")
    out_d = nc.dram_tensor("out", [1], F32, kind="ExternalOutput")

    with tile.TileContext(nc) as tc:
        with tc.tile_pool(name="cpool", bufs=1) as cpool, \
             tc.tile_pool(name="work", bufs=1) as work, \
             tc.tile_pool(name="hpool", bufs=1) as hpool, \
             tc.tile_pool(name="wpool", bufs=1) as wpool, \
             tc.tile_pool(name="psum", bufs=1, space=bass.MemorySpace.PSUM) as psum, \
             tc.tile_pool(name="dpool", bufs=1, space="DRAM") as dpool:

            ones_col = cpool.tile([128, 1], BF16, name="ones_col")
            nc.vector.memset(ones_col[:], 1.0)

            parts = cpool.tile([128, 4], F32, name="parts")
            nc.vector.memset(parts[:], 0.0)
            nc.sync.dma_start(parts[0:1, 3:4], ins["ctrl8"][:])

            x0T_sb, W_sb, wf_sb = {}, {}, {}
            for g in TOWERS:
                x0T_sb[g] = cpool.tile([D, NB], BF16, name=f"x0T_sb_{g}")
                nc.sync.dma_start(x0T_sb[g][:], ins[f"x0T_{g}"][:])
                W_sb[g] = cpool.tile([128, L * D], BF16, name=f"W_sb_{g}")
                nc.sync.dma_start(W_sb[g][:], ins[f"W_{g}"].rearrange("p l d -> p (l d)"))
                wf_sb[g] = cpool.tile([128, L * 2], BF16, name=f"wf_sb_{g}")
                nc.sync.dma_start(wf_sb[g][:], ins[f"wf_{g}"].rearrange("p l j -> p (l j)"))

            def produce_block(g, l, xT_ap):
                """From xT [d, own-nodes] compute h-block (+s2 col) payload and
                s1 row for layer l; kick off the AllGather. Returns (s1row, cco)."""
                pay = work.tile([128, MC * 129], BF16, tag="pay", bufs=2,
                                name=f"pay_{g}{l}")
                for c in range(MC):
                    ph = psum.tile([128, 129], F32, tag="ph", bufs=2,
                                   name=f"ph_{g}{l}{c}")
                    nc.tensor.matmul(ph[:, 0:128], xT_ap[:, c * 128:(c + 1) * 128],
                                     W_sb[g][:, l * D:(l + 1) * D],
                                     start=True, stop=True)
                    nc.tensor.matmul(ph[:, 128:129], xT_ap[:, c * 128:(c + 1) * 128],
                                     wf_sb[g][:, 2 * l + 1:2 * l + 2],
                                     start=True, stop=True)
                    nc.vector.tensor_copy(pay[:, c * 129:(c + 1) * 129], ph[:])
                ps1 = psum.tile([1, NB], F32, tag="ps1", bufs=1, name=f"ps1_{g}{l}")
                nc.tensor.matmul(ps1[:], wf_sb[g][:, 2 * l:2 * l + 1], xT_ap[:],
                                 start=True, stop=True)
                s1row = work.tile([1, NB], BF16, tag="s1row", bufs=3,
                                  name=f"s1row_{g}{l}")
                nc.vector.tensor_copy(s1row[:], ps1[:])

                cci = dpool.tile([128, MC * 129], BF16, tag=f"cci_{g}", bufs=2,
                                 name=f"cci_{g}{l}")
                cco = dpool.tile([NCORES, 128, MC * 129], BF16, tag=f"cco_{g}",
                                 bufs=2, addr_space="Shared", name=f"cco_{g}{l}")
                nc.sync.dma_start(cci[:], pay[:])
                nc.gpsimd.collective_compute(
                    "AllGather", ALU.bypass, replica_groups=RG,
                    ins=[cci[:].opt()], outs=[cco[:].opt()])
                return s1row, cco

            NDVE = EGRP - NPRELU

            def chunkwork(g, l, s1row, cco, ti):
                """Softmax over all 4096 keys for this core's 512 queries of
                tower g, layer l, plus the attention matmul; returns new
                xT [d, own-nodes] (None on the last layer)."""
                last = (l == L - 1)
                h_sb = hpool.tile([128, KC * 129], BF16, tag="h", bufs=3,
                                  name=f"h_sb_{g}{l}")
                nc.sync.dma_start(h_sb.rearrange("p (g x) -> p g x", g=NCORES), cco.rearrange("g p x -> p g x"))
                h3 = h_sb.rearrange("p (k j) -> p k j", j=129)  # k = 4*g + c order preserved
                s2f = work.tile([128, KC], F32, tag="s2f", bufs=2, name=f"s2f_{g}{l}")
                nc.vector.tensor_copy(s2f[:], h3[:, :, 128:129])
                s1d = dpool.tile([NB], BF16, tag="s1d", bufs=2, name=f"s1d_{g}{l}")
                nc.sync.dma_start(s1d[:], s1row[:])
                s1b = work.tile([128, NB], BF16, tag="s1b", bufs=2, name=f"s1b_{g}{l}")
                nc.sync.dma_start(s1b[:], s1d.rearrange("(a f) -> a f", a=1).broadcast_to([128, NB]))

                px = psum.tile([128, NB], F32, tag="px", bufs=2, name=f"px_{g}{l}")
                pden4 = psum.tile([128, NB], F32, tag="pden4", bufs=2,
                                  name=f"pden4_{g}{l}")
                for g0 in range(0, KC, EGRP):
                    wgrp = wpool.tile([128, EGRP * NB], BF16, tag="wgrp", bufs=3,
                                      name=f"wgrp_{g}{l}{g0}")
                    zgrp = wpool.tile([128, NDVE * NB], BF16, tag="zgrp", bufs=3,
                                      name=f"zgrp_{g}{l}{g0}")
                    for ko in range(NDVE):
                        kc = g0 + ko
                        nc.vector.tensor_scalar(zgrp[:, ko * NB:(ko + 1) * NB],
                                                s1b[:], s2f[:, kc:kc + 1],
                                                None, ALU.add)
                    nc.vector.scalar_tensor_tensor(wgrp[:, 0:NDVE * NB], zgrp[:],
                                                   0.01, zgrp[:], ALU.mult, ALU.max)
                    for ko in range(NDVE, EGRP):
                        kc = g0 + ko
                        nc.scalar.activation(wgrp[:, ko * NB:(ko + 1) * NB], s1b[:],
                                             AF.Prelu, bias=s2f[:, kc:kc + 1],
                                             scale=1.0, alpha=0.01)
                    egrp = wpool.tile([128, EGRP * NB], BF16, tag="egrp", bufs=4,
                                      name=f"egrp_{g}{l}{g0}")
                    nc.scalar.activation(egrp[:], wgrp[:], AF.Exp)
                    for ko in range(EGRP):
                        kc = g0 + ko
                        esl = slice(ko * NB, (ko + 1) * NB)
                        hsl = slice(kc * 129, kc * 129 + 128)
                        nc.tensor.matmul(px[:], h_sb[:, hsl], egrp[:, esl],
                                         start=(kc == 0), stop=(kc == KC - 1))
                        j = ko % 4
                        nc.tensor.matmul(pden4[32 * j:32 * j + 1, :], ones_col[:],
                                         egrp[:, esl], start=(kc < 4),
                                         stop=(kc >= KC - 4),
                                         tile_position=(0, 32 * j))

                # 1/den: bounce through DRAM into [128, 4] so reciprocal runs
                # 128 lanes wide. All hops on the idle gpsimd DMA queue to
                # keep this latency chain out of the busy bulk-DMA queues.
                denf = work.tile([128, NB], F32, tag="denf", bufs=2, name=f"denf_{g}{l}")
                nc.scalar.activation(denf[:], pden4[:], AF.Copy)
                dscr = dpool.tile([4, NB], F32, tag="dscr", bufs=2, name=f"dscr_{g}{l}")
                for j in range(4):
                    nc.gpsimd.dma_start(dscr[j], denf[32 * j:32 * j + 1, :])
                denc4 = work.tile([128, 16], F32, tag="denc4", bufs=2,
                                  name=f"denc4_{g}{l}")
                for j in range(4):
                    nc.gpsimd.dma_start(denc4.rearrange("p (c j) -> p c j", j=4)[:, :, j],
                                        dscr[j].rearrange("(c p) -> p c", p=128))
                den_c = work.tile([128, 4], F32, tag="den_c", bufs=2, name=f"denc_{g}{l}")
                nc.vector.tensor_reduce(den_c[:], denc4.rearrange("p (c j) -> p c j", j=4),
                                        AX.X, ALU.add)
                rec_c = work.tile([128, 4], F32, tag="rec_c", bufs=2, name=f"rec_{g}{l}")
                nc.vector.reciprocal(rec_c[:], den_c[:])
                dscr2 = dpool.tile([NB], F32, tag="dscr2", bufs=2, name=f"dscr2_{g}{l}")
                nc.gpsimd.dma_start(dscr2.rearrange("(c p) -> p c", p=128), rec_c[:])
                db = work.tile([128, NB], F32, tag="db", bufs=2, name=f"db_{g}{l}")
                nc.sync.dma_start(db[:], dscr2.rearrange("(a f) -> a f", a=1).broadcast_to([128, NB]))
                xT = work.tile([128, NB], BF16, tag="xT", bufs=2, name=f"xT_{g}{l}")
                nc.vector.tensor_tensor(xT[:], px[:], db[:], ALU.mult)
                if last:
                    nc.vector.tensor_reduce(parts[:, ti:ti + 1], xT[:], AX.X, ALU.add)
                    return None
                return xT

            s1rows, ccos = {}, {}
            for g in TOWERS:
                s1rows[g], ccos[g] = produce_block(g, 0, x0T_sb[g])
            for l in range(L):
                for ti, g in enumerate(TOWERS):
                    xT = chunkwork(g, l, s1rows[g], ccos[g], ti)
                    if l < L - 1:
                        s1rows[g], ccos[g] = produce_block(g, l + 1, xT)

            # ---- tail: gather partials, reduce, replicated MLP head ----
            red_in = dpool.tile([128, 4], F32, name="red_in")
            red_out = dpool.tile([NCORES, 128, 4], F32, addr_space="Shared",
                                 name="red_out")
            nc.sync.dma_start(red_in[:], parts[:])
            nc.gpsimd.collective_compute(
                "AllGather", ALU.bypass, replica_groups=RG,
                ins=[red_in[:].opt()], outs=[red_out[:].opt()])
            red_sb = work.tile([128, NCORES * 4], F32, name="red_sb")
            for gg in range(NCORES):
                nc.sync.dma_start(red_sb[:, gg * 4:(gg + 1) * 4], red_out[gg])
            xcol = work.tile([128, 4], F32, name="xcol")
            nc.vector.tensor_reduce(xcol[:], red_sb.rearrange("p (g t) -> p t g", t=4),
                                    AX.X, ALU.add)

            wh0 = cpool.tile([128, 4, HP], F32, name="wh0")
            wh1 = cpool.tile([128, 4, HP], F32, name="wh1")
            wpj = cpool.tile([128, 4, D], F32, name="wpj")
            bh0 = cpool.tile([128, 4], F32, name="bh0s")
            bh1 = cpool.tile([128, 4], F32, name="bh1s")
            bpj = cpool.tile([128, 1], F32, name="bpjs")
            wo = cpool.tile([128, 1], F32, name="wos")
            bo = cpool.tile([1, 1], F32, name="bos")
            nc.sync.dma_start(wh0[:], ins["Wh0"][:])
            nc.sync.dma_start(wh1[:], ins["Wh1"][:])
            nc.sync.dma_start(wpj[:], ins["Wpj"][:])
            nc.sync.dma_start(bh0[:], ins["bh0"][:])
            nc.sync.dma_start(bh1[:], ins["bh1"][:])
            nc.sync.dma_start(bpj[:], ins["bpj"][:])
            nc.sync.dma_start(wo[:], ins["Wo"][:])
            nc.sync.dma_start(bo[:], ins["bo"].rearrange("(a b) -> a b", a=1))

            def head_layer(y_in, w_sb, b_sb, jo, name):
                y_out = work.tile([128, jo], F32, tag="ycol", bufs=2, name=name)
                ji = 4
                for j in range(jo):
                    py = psum.tile([128, 1], F32, tag="ph", bufs=2,
                                   name=f"py_{name}{j}")
                    for i in range(ji):
                        nc.tensor.matmul(py[:], w_sb[:, i, j * 128:(j + 1) * 128],
                                         y_in[:, i:i + 1],
                                         start=(i == 0), stop=(i == ji - 1))
                    nc.scalar.activation(y_out[:, j:j + 1], py[:], AF.Gelu,
                                         bias=b_sb[:, j:j + 1])
                return y_out

            y1 = head_layer(xcol, wh0, bh0, 4, "y1")
            y2 = head_layer(y1, wh1, bh1, 4, "y2")
            y3 = head_layer(y2, wpj, bpj, 1, "y3")
            po = psum.tile([1, 1], F32, tag="ps1", bufs=1, name="po")
            nc.tensor.matmul(po[:], wo[:], y3[:], start=True, stop=True)
            o_sb = work.tile([1, 1], F32, name="o_sb")
            nc.scalar.activation(o_sb[:], po[:], AF.Identity, bias=bo[0:1, 0:1])
            nc.sync.dma_start(out_d[:], o_sb[:])

    nc.compile()
    return nc


def _prep_in_maps(inputs):
    bf16 = ml_dtypes.bfloat16
    f32 = np.float32

    def bf(a):
        return np.ascontiguousarray(np.asarray(a, dtype=f32)).astype(bf16)

    common = {}
    percore = [dict() for _ in range(NCORES)]
    for g in TOWERS:
        x0 = np.concatenate([np.asarray(inputs[f"{g}_a"], f32),
                             np.asarray(inputs[f"{g}_aim"], f32),
                             np.asarray(inputs[f"{g}_q"], f32)[:, None]], axis=1)
        x0T = np.ascontiguousarray(x0.T)          # [D, N]
        W = np.asarray(inputs[f"{g}_W"], f32)     # [L, D, D]
        att = np.asarray(inputs[f"{g}_att"], f32)  # [L, 2D]
        wf = np.stack([np.einsum("lij,lj->li", W, att[:, :D]),
                       np.einsum("lij,lj->li", W, att[:, D:])], axis=-1)  # [L,D,2]
        common[f"W_{g}"] = bf(W.transpose(1, 0, 2))        # [D(in), L, D(out)]
        common[f"wf_{g}"] = bf(wf.transpose(1, 0, 2))      # [D(in), L, 2]
        for c in range(NCORES):
            percore[c][f"x0T_{g}"] = bf(x0T[:, c * NB:(c + 1) * NB])

    common["ctrl8"] = np.asarray(inputs["control"], f32) / NCORES
    H = 3 * D + 1
    pad2 = lambda a, r, c: np.pad(np.asarray(a, f32), ((0, r - a.shape[0]), (0, c - a.shape[1])))
    pad1 = lambda a, r: np.pad(np.asarray(a, f32), (0, r - a.shape[0]))
    ptile = lambda a: np.ascontiguousarray(a.reshape(4, 128, -1).transpose(1, 0, 2))
    common["Wh0"] = ptile(pad2(inputs["W_h0"], HP, HP))      # [128, 4, HP]
    common["bh0"] = np.ascontiguousarray(pad1(inputs["b_h0"], HP).reshape(4, 128).T)
    common["Wh1"] = ptile(pad2(inputs["W_h1"], HP, HP))
    common["bh1"] = np.ascontiguousarray(pad1(inputs["b_h1"], HP).reshape(4, 128).T)
    common["Wpj"] = ptile(pad2(inputs["W_proj"], HP, D))     # [128, 4, D]
    common["bpj"] = np.asarray(inputs["b_proj"], f32).reshape(128, 1)
    common["Wo"] = np.asarray(inputs["W_out"], f32)
    common["bo"] = np.asarray(inputs["b_out"], f32)

    return [dict(common, **percore[c]) for c in range(NCORES)]


def kernel(**inputs):
    global LAST_RESULT, _CACHED
    from concourse.bass_utils import run_bass_kernel_spmd

    if _CACHED is None:
        _CACHED = _build()
    nc = _CACHED
    in_maps = _prep_in_maps(inputs)
    res = run_bass_kernel_spmd(nc, in_maps, core_ids=list(range(NCORES)))
    LAST_RESULT = res
    return np.asarray(res.results[0]["out"], np.float32)
